# revision 14
# baseline (speedup 1.0000x reference)
"""MultiHeadAttention TRN2 Bass kernel.

Problem: B=4, S=2048, D=768, H=12 heads (DK=64).
Sharding: 8 cores = (batch b in 0..3) x (head-half in 0..1); each core
computes 6 heads of one batch element end-to-end (tensor-parallel over
heads within a batch). Host pre-transposes activations to [D, S] (and
casts to bf16 in the default fast path), slices projection weights per
head-half, and sums the two partial outputs per batch (+ bv@Wo + bo
correction, exact because softmax rows sum to 1).

Key compaction: the boolean mask drops ~half the key positions, so the
host gathers the unmasked keys to the front of kT/vT (padded with zeros
to a multiple of 128, SK columns total) and sets mv=1 for real keys /
0 for padding. The padded columns flow through the same mask-fold math
(vh_aug *= mv) the full kernel used, so the result is exact while the
score/exp/attn@V/k-proj/v-proj work drops to SK/S of the dense cost.

On-core math:
  qh^T[384, S]: lhsT=Wq tile [Din,dout], rhs=q^T tile [Din,s] (+bq in drain)
  kh^T likewise over SK; vh natural [SK, 390] via lhsT=v^T tile, rhs=Wv:
    vh_aug[s, 65j..65j+64] = [m(s)*vh_head_j(s, :), m(s)]  (mask fold)
  S^T[k, q] = kh_head^T.T @ qh_head^T  (contraction d=64)
  P^T = exp(S^T * 0.125)               (ACT, fused scale, no max-sub)
  ctx_aug^T[0:65, q] += vh_aug_j[kc].T @ P^T[kc]  over k-chunks
    rows 0..63 = unnormalized ctx^T, row 64 = softmax denominator
  rs = recip-approx(denom); bcast via gpsimd partition_broadcast;
  cn = ctx^T * rs   (drains deferred one (h,qb) iteration so the PE
                     pipeline never waits on the reciprocal chain)
  out[q, 768] = sum_dt cn[dt].T @ Wo tiles  (per 128-q chunk, split in
  two PSUM-bank-sized halves so transient PSUM stays at 1 bank)

Bandwidth notes: every input stream is fetched with ONE 3D-strided DMA
per chunk ([768, w] DRAM block -> [128, 6*w] SBUF tile, p-c-s order) so
phase 1 issues ~13 DMA instructions instead of ~45 (each costs ~600ns
of queue issue time). Projection dt-chains are interleaved across three
PSUM banks to keep the PE pipeline full.
"""

import os
import sys
import types
from contextlib import ExitStack

import ml_dtypes
import numpy as np

import concourse.bacc as bacc
import concourse.bass as bass
import concourse.mybir as mybir
import concourse.tile as tile
from concourse import bass_utils
from concourse.bass import ts, ds

F32 = mybir.dt.float32
F32R = mybir.dt.float32r
BF16 = mybir.dt.bfloat16

D = 768        # model dim
DH = 384       # per-core head dim (6 heads x 64)
HPC = 6        # heads per core
VW = HPC * 65  # vh_aug free width (390)


def _chunks(total, w=512):
    out = []
    off = 0
    while off < total:
        cw = min(w, total - off)
        out.append((off, cw))
        off += cw
    return out


def build_nc(S=2048, SK=1152, bf16=True):
    nc = bacc.Bacc("TRN2", target_bir_lowering=False, debug=False)

    MMD = BF16 if bf16 else F32R    # matmul operand dtype
    QBW = min(512, S)               # attention q-block width
    NKT = SK // 128                 # 128-wide k tiles
    NQB = S // QBW                  # q blocks
    KCH = _chunks(SK)               # k/v projection chunks (<=512 wide)
    QCH = _chunks(S)                # q projection chunks (512 wide)

    qT = nc.dram_tensor("qT", [D, S], MMD, kind="ExternalInput").ap()
    kT = nc.dram_tensor("kT", [D, SK], MMD, kind="ExternalInput").ap()
    vT = nc.dram_tensor("vT", [D, SK], MMD, kind="ExternalInput").ap()
    wq = nc.dram_tensor("wq", [D, DH], MMD, kind="ExternalInput").ap()
    wk = nc.dram_tensor("wk", [D, DH], MMD, kind="ExternalInput").ap()
    wv = nc.dram_tensor("wv", [D, DH], MMD, kind="ExternalInput").ap()
    wo = nc.dram_tensor("wo", [DH, D], MMD, kind="ExternalInput").ap()
    bq = nc.dram_tensor("bq", [DH, 1], F32, kind="ExternalInput").ap()
    bk = nc.dram_tensor("bk", [DH, 1], F32, kind="ExternalInput").ap()
    mv = nc.dram_tensor("mv", [SK, 1], F32, kind="ExternalInput").ap()
    out = nc.dram_tensor("out", [S, D], F32, kind="ExternalOutput").ap()

    with tile.TileContext(nc) as tc, ExitStack() as ctx:
        P = 128
        wpool = ctx.enter_context(tc.tile_pool(name="w", bufs=1))
        xin = ctx.enter_context(
            tc.tile_pool(name="xin", bufs=2 * len(KCH) + 1)
        )
        qdef = ctx.enter_context(tc.tile_pool(name="qdef", bufs=3))
        persist = ctx.enter_context(tc.tile_pool(name="persist", bufs=1))
        ppool = ctx.enter_context(tc.tile_pool(name="p", bufs=3))
        small = ctx.enter_context(tc.tile_pool(name="small", bufs=2))
        outp = ctx.enter_context(tc.tile_pool(name="outp", bufs=3))
        psA = ctx.enter_context(tc.tile_pool(name="psA", bufs=2, space="PSUM"))
        psB = ctx.enter_context(tc.tile_pool(name="psB", bufs=4, space="PSUM"))

        def fetch_cpd(dst, src, coff, w, eng, csz=512, nch=6):
            """One 3D DMA: DRAM [nch*P, w] block -> SBUF [P, nch blocks of csz]."""
            d3 = dst[:].rearrange("p (c s) -> p c s", s=csz)[:, :, :w]
            s3 = src[ds(0, nch * P), ds(coff, w)].rearrange(
                "(c p) s -> p c s", p=P
            )
            eng.dma_start(d3, s3)

        # ---- weights + all phase-1 input streams, one DMA per chunk,
        # issued upfront in consumption order across both bulk queues ----
        wk_sb = wpool.tile([P, 6 * DH], MMD, tag="wk")
        wq_sb = wpool.tile([P, 6 * DH], MMD, tag="wq")
        wv_sb = wpool.tile([P, 6 * DH], MMD, tag="wv")
        wo_sb = wpool.tile([P, 3 * D], MMD, tag="wo")
        kx = [xin.tile([P, 6 * 512], MMD, name="xin", tag="xin") for _ in KCH]
        vx = [xin.tile([P, 6 * 512], MMD, name="xin", tag="xin") for _ in KCH]
        q0x = xin.tile([P, 6 * 512], MMD, name="xin", tag="xin")

        fetch_cpd(wk_sb, wk, 0, DH, nc.sync, csz=DH)
        fetch_cpd(kx[0], kT, KCH[0][0], KCH[0][1], nc.gpsimd)
        for ci in range(1, len(KCH)):
            fetch_cpd(kx[ci], kT, KCH[ci][0], KCH[ci][1],
                      nc.sync if ci % 2 else nc.gpsimd)
        fetch_cpd(wv_sb, wv, 0, DH, nc.gpsimd, csz=DH)
        for ci in range(len(KCH)):
            fetch_cpd(vx[ci], vT, KCH[ci][0], KCH[ci][1],
                      nc.sync if ci % 2 == 0 else nc.gpsimd)
        fetch_cpd(wq_sb, wq, 0, DH, nc.gpsimd, csz=DH)
        fetch_cpd(q0x, qT, 0, 512, nc.sync)

        # small tensors on the scalar queue (idle during phase 1)
        bq_sb = wpool.tile([P, 3], F32, tag="bq")
        bk_sb = wpool.tile([P, 3], F32, tag="bk")
        mv_sb = wpool.tile([P, NKT], F32, tag="mv")
        nc.scalar.dma_start(
            bq_sb[:].rearrange("p (t u) -> p t u", u=1),
            bq[ds(0, DH), :].rearrange("(t p) u -> p t u", p=P),
        )
        nc.scalar.dma_start(
            bk_sb[:].rearrange("p (t u) -> p t u", u=1),
            bk[ds(0, DH), :].rearrange("(t p) u -> p t u", p=P),
        )
        nc.scalar.dma_start(
            mv_sb[:].rearrange("p (t u) -> p t u", u=1),
            mv[ds(0, SK), :].rearrange("(t p) u -> p t u", p=P),
        )
        ones6 = wpool.tile([P, HPC], F32, tag="ones6")
        nc.vector.memset(ones6[:], 1.0)

        # ---- persistent activations ----
        khT = [persist.tile([P, SK], MMD, name=f"khT{t}", tag=f"khT{t}") for t in range(3)]
        qhT = [persist.tile([P, S], MMD, name=f"qhT{t}", tag=f"qhT{t}") for t in range(3)]
        vh = [persist.tile([P, VW], MMD, name=f"vh{st}", tag=f"vh{st}") for st in range(NKT)]
        cn = [persist.tile([P, S], MMD, name=f"cn{t}", tag=f"cn{t}") for t in range(3)]

        # ---- phase 1: projections. dt-chains interleave across three PSUM
        # banks; q chunks 1.. are deferred into the attention stream ----
        def proj_chunk(xt, wsb, bsb, dst, off, w):
            ps3 = [psB.tile([P, 512], F32, name="psB", tag="psB") for _ in range(3)]
            for c in range(6):
                for dt in range(3):
                    nc.tensor.matmul(
                        ps3[dt][:, :w],
                        lhsT=wsb[:, ds(c * DH + dt * P, P)],
                        rhs=xt[:, ds(c * 512, w)],
                        start=(c == 0),
                        stop=(c == 5),
                    )
            for dt in range(3):
                nc.vector.tensor_scalar_add(
                    out=dst[dt][:, ds(off, w)], in0=ps3[dt][:, :w],
                    scalar1=bsb[:, ds(dt, 1)],
                )

        for ci, (off, w) in enumerate(KCH):
            proj_chunk(kx[ci], wk_sb, bk_sb, khT, off, w)

        # v-projection: st chains pair-interleaved across two PSUM banks
        def vproj_sts(vt, base_st, sjs):
            pss = [psB.tile([P, 512], F32, name="psB", tag="psB") for _ in sjs]
            for c in range(6):
                for i, sj in enumerate(sjs):
                    nc.tensor.matmul(
                        pss[i][:, :DH],
                        lhsT=vt[:, ds(c * 512 + sj * P, P)],
                        rhs=wv_sb[:, ds(c * DH, DH)],
                        start=(c == 0),
                        stop=(c == 5),
                    )
            for i, sj in enumerate(sjs):
                st = base_st + sj
                vh3 = vh[st].rearrange("p (h c) -> p h c", c=65)
                nc.vector.tensor_scalar_mul(
                    out=vh3[:, :, 0:64],
                    in0=pss[i][:, :DH].rearrange("p (h c) -> p h c", c=64),
                    scalar1=mv_sb[:, ds(st, 1)],
                )
                nc.vector.tensor_scalar_mul(
                    out=vh3[:, :, 64:65],
                    in0=ones6[:].rearrange("p (h c) -> p h c", c=1),
                    scalar1=mv_sb[:, ds(st, 1)],
                )

        for ci, (off, w) in enumerate(KCH):
            nst = w // P
            for s0 in range(0, nst, 2):
                vproj_sts(vx[ci], off // P, list(range(s0, min(s0 + 2, nst))))

        # q-projection: first q-block now, rest deferred into phase 2
        proj_chunk(q0x, wq_sb, bq_sb, qhT, 0, QCH[0][1])
        pend_qproj = [
            (ch, dt) for ch in range(1, len(QCH)) for dt in range(3)
        ]
        qproj_xt = {}

        def issue_qdef_dma(ch):
            qx = qdef.tile([P, 6 * 512], MMD, name="qx", tag="qx")
            qproj_xt[ch] = qx
            fetch_cpd(qx, qT, QCH[ch][0], 512, nc.gpsimd)

        # ---- phase 2: attention, head-pair steps ----
        # Each step handles BOTH heads of a pair for one k-chunk: the two
        # scores matmuls live in disjoint PE row groups (base partition 0
        # and 64) so they run concurrently, and share one [128,1024] PSUM
        # tile (head A in cols 0:512, head B in 512:1024) -> one exp per
        # step. Scores run 2 steps ahead of attn@V. Filler work is placed
        # so transient PSUM use never exceeds the free banks: drains right
        # at group start (kc 1,2) free the previous ctx pair early, then
        # q-proj (kc 4,6) and O-proj halves (odd kc>=3) fill PE slack.
        hq = [(pr, qb) for qb in range(NQB) for pr in range(3)]
        steps = [(pr, qb, kc) for (pr, qb) in hq for kc in range(NKT)]
        drain_slots = (1, 2)
        qproj_slots = (4, 6)
        oproj_slots = tuple(k for k in range(3, NKT) if k % 2 == 1)

        ctx_ps = {}
        st_ps = {}
        ot_map = {}

        def scores(pr, qb, kc):
            ps = psA.tile([P, 1024], F32, name="psA", tag="psA")
            for hh in range(2):
                nc.tensor.matmul(
                    ps[:, ts(hh, 512)],
                    lhsT=khT[pr][64 * hh : 64 * hh + 64, ts(kc, P)],
                    rhs=qhT[pr][64 * hh : 64 * hh + 64, ts(qb, QBW)],
                    start=True,
                    stop=True,
                )
            st_ps[(pr, qb, kc)] = ps

        def attnv(pr, qb, kc, pt):
            for hh in range(2):
                h = 2 * pr + hh
                nc.tensor.matmul(
                    ctx_ps[(h, qb)][0:65, :],
                    lhsT=vh[kc][:, ds(65 * h, 65)],
                    rhs=pt[:, ts(hh, 512)],
                    start=(kc == 0),
                    stop=(kc == NKT - 1),
                )

        def drain_many(items):
            """Normalize + store ctx for finished (h, qb) pairs, with the
            per-engine stages interleaved so the DVE/gpsimd chains of the
            different heads overlap instead of serializing."""
            st = []
            for h, qb in items:
                cps = ctx_ps.pop((h, qb))
                # reciprocal_approx_fast can't read partition-offset APs
                # (the custom-DVE encoding drops the partition base), so
                # stage the denominator row at partition 0 first.
                dcp = small.tile([1, QBW], F32, name="dcp", tag="dcp")
                nc.vector.tensor_copy(dcp[:], cps[64:65, :])
                st.append((h, qb, cps, dcp))
            rss = []
            for h, qb, cps, dcp in st:
                rs = small.tile([1, QBW], F32, name="rs", tag="rs")
                nc.vector.reciprocal_approx_fast(rs[:], dcp[:])
                rss.append(rs)
            bcss = []
            for (h, qb, cps, dcp), rs in zip(st, rss):
                bcs = small.tile([64, QBW], F32, name="bcs", tag="bcs")
                nc.gpsimd.partition_broadcast(bcs[:], rs[:])
                bcss.append(bcs)
            for (h, qb, cps, dcp), bcs in zip(st, bcss):
                dt, pb = h // 2, 64 * (h % 2)
                if pb == 0:
                    nc.vector.tensor_tensor(
                        out=cn[dt][0:64, ts(qb, QBW)],
                        in0=cps[0:64, :],
                        in1=bcs[:],
                        op=mybir.AluOpType.mult,
                    )
                else:
                    tmp = small.tile([64, QBW], MMD, name="tmp", tag="tmp")
                    nc.vector.tensor_tensor(
                        out=tmp[:], in0=cps[0:64, :], in1=bcs[:],
                        op=mybir.AluOpType.mult,
                    )
                    nc.sync.dma_start(cn[dt][64:128, ts(qb, QBW)], tmp[:])

        def oproj_half(qc, n0, nw, pool=None):
            if pool is None:
                ps = psB.tile([P, 512], F32, name="psB", tag="psB")
            else:
                ps = pool.tile([P, 1024], F32, name="psA", tag="psA")
            for dt in range(3):
                nc.tensor.matmul(
                    ps[:, :nw],
                    lhsT=cn[dt][:, ts(qc, P)],
                    rhs=wo_sb[:, ds(dt * D + n0, nw)],
                    start=(dt == 0),
                    stop=(dt == 2),
                )
            if n0 == 0:
                ot = outp.tile([P, D], F32, name="ot", tag="ot")
                ot_map[qc] = ot
                nc.vector.tensor_copy(ot[:, 0:512], ps[:, :nw])
            else:
                ot = ot_map.pop(qc)
                nc.vector.tensor_copy(ot[:, 512:768], ps[:, :nw])
                nc.sync.dma_start(out[ts(qc, P), :], ot[:])

        def qproj_sub(ch, dt):
            ps = psB.tile([P, 512], F32, name="psB", tag="psB")
            qx = qproj_xt[ch]
            for c in range(6):
                nc.tensor.matmul(
                    ps[:],
                    lhsT=wq_sb[:, ds(c * DH + dt * P, P)],
                    rhs=qx[:, ds(c * 512, 512)],
                    start=(c == 0),
                    stop=(c == 5),
                )
            nc.vector.tensor_scalar_add(
                out=qhT[dt][:, ds(QCH[ch][0], 512)], in0=ps[:],
                scalar1=bq_sb[:, ds(dt, 1)],
            )
            if dt == 2:
                qproj_xt.pop(ch)

        DEPTH = 2
        pend_drain = []
        pend_oproj = []
        for n, (pr, qb, kc) in enumerate(steps):
            grp = n // NKT
            if kc == 0:
                # lazy bulk DMAs: deferred q chunks at groups 0..; wo at 1
                if grp + 1 < len(QCH):
                    issue_qdef_dma(grp + 1)
                if grp == 1:
                    fetch_cpd(wo_sb, wo, 0, D, nc.sync, csz=D, nch=3)
                for hh in range(2):
                    ctx_ps[(2 * pr + hh, qb)] = psB.tile(
                        [P, QBW], F32, name="psB", tag="psB"
                    )[0:65, :]
            if n < DEPTH:
                scores(*steps[n])
            pt = ppool.tile([P, 1024], MMD, name="pt", tag="pt")
            nc.scalar.activation(
                pt[:], st_ps.pop((pr, qb, kc))[:],
                mybir.ActivationFunctionType.Exp, scale=0.125,
            )
            if n + DEPTH < len(steps):
                scores(*steps[n + DEPTH])
            attnv(pr, qb, kc, pt)
            if kc in drain_slots and pend_drain:
                hd, qd = pend_drain.pop(0)
                drain_many([(hd, qd)])
                if hd == HPC - 1:
                    for qc in range(qd * (QBW // P), (qd + 1) * (QBW // P)):
                        pend_oproj.append((qc, 0, 512))
                        pend_oproj.append((qc, 512, 256))
            elif kc in qproj_slots and pend_qproj and grp >= 1:
                qproj_sub(*pend_qproj.pop(0))
            elif kc in oproj_slots and pend_oproj:
                oproj_half(*pend_oproj.pop(0))
            if kc == NKT - 1:
                pend_drain.extend([(2 * pr, qb), (2 * pr + 1, qb)])
        while pend_drain:
            batch, pend_drain = pend_drain[:2], pend_drain[2:]
            drain_many(batch)
            for hd, qd in batch:
                if hd == HPC - 1:
                    for qc in range(qd * (QBW // P), (qd + 1) * (QBW // P)):
                        pend_oproj.append((qc, 0, 512))
                        pend_oproj.append((qc, 512, 256))
        # tail: alternate psA (idle now) and psB so up to 6 halves in flight
        for i, (qc, n0, nw) in enumerate(pend_oproj):
            oproj_half(qc, n0, nw, pool=psA if i % 2 == 0 else None)

    nc.compile()
    return nc


_NC_CACHE = {}


def _get_nc(S, SK, bf16=True):
    key = (S, SK, bf16)
    if key not in _NC_CACHE:
        _NC_CACHE[key] = build_nc(S, SK, bf16)
    return _NC_CACHE[key]


def _install_ntff_hook():
    try:
        mod = types.ModuleType("antenv.axon_hooks")
        state = {"hook": None}
        mod.set_axon_ntff_profile_hook = lambda h: state.__setitem__("hook", h)
        mod.get_axon_ntff_profile_hook = lambda: state["hook"]
        sys.modules["antenv.axon_hooks"] = mod
        from trn_agent_boot.trn_boot import _ntff_profile_via_ctypes

        mod.set_axon_ntff_profile_hook(
            _ntff_profile_via_ctypes("/opt/axon/libaxon_pjrt.so")
        )
        bass_utils.upload_artifacts = lambda tmpdir: "local://" + tmpdir
        return state["hook"] is not None
    except Exception:
        return False


def run_cores(in_maps, S=2048, SK=1152, bf16=True, profile=False):
    nc = _get_nc(S, SK, bf16)
    trace = bool(profile) and _install_ntff_hook()
    res = bass_utils.run_bass_kernel_spmd(
        nc, in_maps, core_ids=list(range(len(in_maps))), trace=trace
    )
    return res


def make_in_maps(q, k, v, mask, Wq, bq, Wk, bk, Wv, Wo, bf16=True):
    B, S, _ = q.shape
    mmd = ml_dtypes.bfloat16 if bf16 else np.float32
    mbool = np.asarray(mask).reshape(B, S)
    keep = [np.nonzero(~mbool[b])[0] for b in range(B)]
    nmax = max((len(ix) for ix in keep), default=1)
    SK = max(128, ((int(nmax) + 127) // 128) * 128)

    qT = np.ascontiguousarray(
        np.asarray(q, np.float32).transpose(0, 2, 1)).astype(mmd)
    kT33 = np.asarray(k, np.float32).transpose(0, 2, 1)
    vT33 = np.asarray(v, np.float32).transpose(0, 2, 1)
    kTc = np.zeros((B, D, SK), np.float32)
    vTc = np.zeros((B, D, SK), np.float32)
    mvec = np.zeros((B, SK), np.float32)
    for b in range(B):
        nb = len(keep[b])
        kTc[b, :, :nb] = kT33[b][:, keep[b]]
        vTc[b, :, :nb] = vT33[b][:, keep[b]]
        mvec[b, :nb] = 1.0
    kTc = kTc.astype(mmd)
    vTc = vTc.astype(mmd)
    Wq, Wk, Wv, Wo = (np.asarray(a, np.float32) for a in (Wq, Wk, Wv, Wo))
    bq, bk = np.asarray(bq, np.float32), np.asarray(bk, np.float32)
    in_maps = []
    for b in range(B):
        for half in range(2):
            hs = slice(DH * half, DH * (half + 1))
            in_maps.append(
                {
                    "qT": qT[b],
                    "kT": kTc[b],
                    "vT": vTc[b],
                    "wq": np.ascontiguousarray(Wq[:, hs]).astype(mmd),
                    "wk": np.ascontiguousarray(Wk[:, hs]).astype(mmd),
                    "wv": np.ascontiguousarray(Wv[:, hs]).astype(mmd),
                    "wo": np.ascontiguousarray(Wo[hs, :]).astype(mmd),
                    "bq": np.ascontiguousarray(bq[hs]).reshape(DH, 1),
                    "bk": np.ascontiguousarray(bk[hs]).reshape(DH, 1),
                    "mv": np.ascontiguousarray(mvec[b]).reshape(-1, 1),
                }
            )
    return in_maps, SK


def kernel(q, k, v, mask, Wq, bq, Wk, bk, Wv, bv, Wo, bo):
    q = np.asarray(q, np.float32)
    B, S, _ = q.shape
    bf16 = os.environ.get("BASS_PRECISE") != "1"
    in_maps, SK = make_in_maps(
        q, k, v, mask, Wq, bq, Wk, bk, Wv, Wo, bf16=bf16
    )
    res = run_cores(
        in_maps, S=S, SK=SK, bf16=bf16,
        profile=os.environ.get("BASS_PROFILE") == "1",
    )
    if os.environ.get("BASS_PROFILE") == "1" and res.exec_time_ns is not None:
        print(f"HW exec time: {res.exec_time_ns} ns")
    cvec = (
        np.asarray(bv, np.float32) @ np.asarray(Wo, np.float32)
        + np.asarray(bo, np.float32)
    )
    out = np.empty((B, S, D), np.float32)
    for b in range(B):
        out[b] = res.results[2 * b]["out"] + res.results[2 * b + 1]["out"] + cvec
    return out


# revision 16
# speedup vs baseline: 1.2535x; 1.2535x over previous
"""MultiHeadAttention TRN2 Bass kernel.

Problem: B=4, S=2048, D=768, H=12 heads (DK=64).
Sharding: 8 cores = (batch b in 0..3) x (head-half in 0..1); each core
computes 6 heads of one batch element end-to-end (tensor-parallel over
heads within a batch). Host pre-transposes activations to [D, S] (and
casts to bf16 in the default fast path), slices projection weights per
head-half, and sums the two partial outputs per batch (+ bv@Wo + bo
correction, exact because softmax rows sum to 1).

Key compaction: the boolean mask drops ~half the key positions, so the
host gathers the unmasked keys to the front of kT/vT (padded with zeros
to a multiple of 128, SK columns total) and sets mv=1 for real keys /
0 for padding. The padded columns flow through the same mask-fold math
(vh_aug *= mv) the full kernel used, so the result is exact while the
score/exp/attn@V/k-proj/v-proj work drops to SK/S of the dense cost.

On-core math:
  qh^T[384, S]: lhsT=Wq tile [Din,dout], rhs=q^T tile [Din,s] (+bq in drain)
  kh^T likewise over SK; vh natural [SK, 390] via lhsT=v^T tile, rhs=Wv:
    vh_aug[s, 65j..65j+64] = [m(s)*vh_head_j(s, :), m(s)]  (mask fold)
  S^T[k, q] = kh_head^T.T @ qh_head^T  (contraction d=64)
  P^T = exp(S^T * 0.125)               (ACT, fused scale, no max-sub)
  ctx_aug^T[0:65, q] += vh_aug_j[kc].T @ P^T[kc]  over k-chunks
    rows 0..63 = unnormalized ctx^T, row 64 = softmax denominator
  rs = recip-approx(denom); bcast via gpsimd partition_broadcast;
  cn = ctx^T * rs   (drains deferred one (h,qb) iteration so the PE
                     pipeline never waits on the reciprocal chain)
  out[q, 768] = sum_dt cn[dt].T @ Wo tiles  (per 128-q chunk, split in
  two PSUM-bank-sized halves so transient PSUM stays at 1 bank)

Bandwidth notes: every input stream is fetched with ONE 3D-strided DMA
per chunk ([768, w] DRAM block -> [128, 6*w] SBUF tile, p-c-s order) so
phase 1 issues ~13 DMA instructions instead of ~45 (each costs ~600ns
of queue issue time). Projection dt-chains are interleaved across three
PSUM banks to keep the PE pipeline full.
"""

import os
import sys
import types
from contextlib import ExitStack

import ml_dtypes
import numpy as np

import concourse.bacc as bacc
import concourse.bass as bass
import concourse.mybir as mybir
import concourse.tile as tile
from concourse import bass_utils
from concourse.bass import ts, ds

F32 = mybir.dt.float32
F32R = mybir.dt.float32r
BF16 = mybir.dt.bfloat16

D = 768        # model dim
DH = 384       # per-core head dim (6 heads x 64)
HPC = 6        # heads per core
VW = HPC * 65  # vh_aug free width (390)


def _chunks(total, w=512):
    out = []
    off = 0
    while off < total:
        cw = min(w, total - off)
        out.append((off, cw))
        off += cw
    return out


def build_nc(S=2048, SK=1152, bf16=True):
    nc = bacc.Bacc("TRN2", target_bir_lowering=False, debug=False)

    MMD = BF16 if bf16 else F32R    # matmul operand dtype
    QBW = min(512, S)               # attention q-block width
    NKT = SK // 128                 # 128-wide k tiles
    NQB = S // QBW                  # q blocks
    KCH = _chunks(SK)               # k/v projection chunks (<=512 wide)
    QCH = _chunks(S)                # q projection chunks (512 wide)

    qT = nc.dram_tensor("qT", [D, S], MMD, kind="ExternalInput").ap()
    kT = nc.dram_tensor("kT", [D, SK], MMD, kind="ExternalInput").ap()
    vT = nc.dram_tensor("vT", [D, SK], MMD, kind="ExternalInput").ap()
    wq = nc.dram_tensor("wq", [D, DH], MMD, kind="ExternalInput").ap()
    wk = nc.dram_tensor("wk", [D, DH], MMD, kind="ExternalInput").ap()
    wv = nc.dram_tensor("wv", [D, DH], MMD, kind="ExternalInput").ap()
    wo = nc.dram_tensor("wo", [DH, D], MMD, kind="ExternalInput").ap()
    bq = nc.dram_tensor("bq", [DH, 1], F32, kind="ExternalInput").ap()
    bk = nc.dram_tensor("bk", [DH, 1], F32, kind="ExternalInput").ap()
    mv = nc.dram_tensor("mv", [SK, 1], F32, kind="ExternalInput").ap()
    out = nc.dram_tensor("out", [S, D], F32, kind="ExternalOutput").ap()

    with tile.TileContext(nc) as tc, ExitStack() as ctx:
        P = 128
        wpool = ctx.enter_context(tc.tile_pool(name="w", bufs=1))
        xin = ctx.enter_context(
            tc.tile_pool(name="xin", bufs=2 * len(KCH) + 1)
        )
        qdef = ctx.enter_context(tc.tile_pool(name="qdef", bufs=3))
        persist = ctx.enter_context(tc.tile_pool(name="persist", bufs=1))
        ppool = ctx.enter_context(tc.tile_pool(name="p", bufs=3))
        small = ctx.enter_context(tc.tile_pool(name="small", bufs=2))
        outp = ctx.enter_context(tc.tile_pool(name="outp", bufs=3))
        psA = ctx.enter_context(tc.tile_pool(name="psA", bufs=2, space="PSUM"))
        psB = ctx.enter_context(tc.tile_pool(name="psB", bufs=4, space="PSUM"))

        def fetch_cpd(dst, src, coff, w, eng, csz=512, nch=6):
            """One 3D DMA: DRAM [nch*P, w] block -> SBUF [P, nch blocks of csz]."""
            d3 = dst[:].rearrange("p (c s) -> p c s", s=csz)[:, :, :w]
            s3 = src[ds(0, nch * P), ds(coff, w)].rearrange(
                "(c p) s -> p c s", p=P
            )
            eng.dma_start(d3, s3)

        # ---- weights + all phase-1 input streams, one DMA per chunk,
        # issued upfront in consumption order across both bulk queues ----
        wk_sb = wpool.tile([P, 6 * DH], MMD, tag="wk")
        wq_sb = wpool.tile([P, 6 * DH], MMD, tag="wq")
        wv_sb = wpool.tile([P, 6 * DH], MMD, tag="wv")
        wo_sb = wpool.tile([P, 3 * D], MMD, tag="wo")
        kx = [xin.tile([P, 6 * 512], MMD, name="xin", tag="xin") for _ in KCH]
        vx = [xin.tile([P, 6 * 512], MMD, name="xin", tag="xin") for _ in KCH]
        q0x = xin.tile([P, 6 * 512], MMD, name="xin", tag="xin")

        def fetch_split(dst, src, coff, w):
            """Bulk chunk fetch: one 2D DMA per 128-row block, alternating
            queues, so the transfer spreads across DMA engines."""
            for c in range(6):
                (nc.sync if c % 2 == 0 else nc.gpsimd).dma_start(
                    dst[:, ds(c * 512, w)], src[ts(c, P), ds(coff, w)]
                )

        fetch_cpd(wk_sb, wk, 0, DH, nc.sync, csz=DH)
        fetch_cpd(wv_sb, wv, 0, DH, nc.gpsimd, csz=DH)
        for ci in range(len(KCH)):
            fetch_split(kx[ci], kT, KCH[ci][0], KCH[ci][1])
        for ci in range(len(KCH)):
            fetch_split(vx[ci], vT, KCH[ci][0], KCH[ci][1])
        fetch_cpd(wq_sb, wq, 0, DH, nc.gpsimd, csz=DH)
        fetch_split(q0x, qT, 0, 512)

        # small tensors on the scalar queue (idle during phase 1)
        bq_sb = wpool.tile([P, 3], F32, tag="bq")
        bk_sb = wpool.tile([P, 3], F32, tag="bk")
        mv_sb = wpool.tile([P, NKT], F32, tag="mv")
        nc.scalar.dma_start(
            bq_sb[:].rearrange("p (t u) -> p t u", u=1),
            bq[ds(0, DH), :].rearrange("(t p) u -> p t u", p=P),
        )
        nc.scalar.dma_start(
            bk_sb[:].rearrange("p (t u) -> p t u", u=1),
            bk[ds(0, DH), :].rearrange("(t p) u -> p t u", p=P),
        )
        nc.scalar.dma_start(
            mv_sb[:].rearrange("p (t u) -> p t u", u=1),
            mv[ds(0, SK), :].rearrange("(t p) u -> p t u", p=P),
        )
        ones6 = wpool.tile([P, HPC], F32, tag="ones6")
        nc.vector.memset(ones6[:], 1.0)

        # ---- persistent activations ----
        khT = [persist.tile([P, SK], MMD, name=f"khT{t}", tag=f"khT{t}") for t in range(3)]
        qhT = [persist.tile([P, S], MMD, name=f"qhT{t}", tag=f"qhT{t}") for t in range(3)]
        vh = [persist.tile([P, VW], MMD, name=f"vh{st}", tag=f"vh{st}") for st in range(NKT)]
        cn = [persist.tile([P, S], MMD, name=f"cn{t}", tag=f"cn{t}") for t in range(3)]

        # ---- phase 1: projections. dt-chains interleave across three PSUM
        # banks; q chunks 1.. are deferred into the attention stream ----
        def proj_chunk(xt, wsb, bsb, dst, off, w):
            ps3 = [psB.tile([P, 512], F32, name="psB", tag="psB") for _ in range(3)]
            for c in range(6):
                for dt in range(3):
                    nc.tensor.matmul(
                        ps3[dt][:, :w],
                        lhsT=wsb[:, ds(c * DH + dt * P, P)],
                        rhs=xt[:, ds(c * 512, w)],
                        start=(c == 0),
                        stop=(c == 5),
                    )
            for dt in range(3):
                nc.vector.tensor_scalar_add(
                    out=dst[dt][:, ds(off, w)], in0=ps3[dt][:, :w],
                    scalar1=bsb[:, ds(dt, 1)],
                )

        for ci, (off, w) in enumerate(KCH):
            proj_chunk(kx[ci], wk_sb, bk_sb, khT, off, w)

        # v-projection: st chains pair-interleaved across two PSUM banks
        def vproj_sts(vt, base_st, sjs):
            pss = [psB.tile([P, 512], F32, name="psB", tag="psB") for _ in sjs]
            for c in range(6):
                for i, sj in enumerate(sjs):
                    nc.tensor.matmul(
                        pss[i][:, :DH],
                        lhsT=vt[:, ds(c * 512 + sj * P, P)],
                        rhs=wv_sb[:, ds(c * DH, DH)],
                        start=(c == 0),
                        stop=(c == 5),
                    )
            for i, sj in enumerate(sjs):
                st = base_st + sj
                vh3 = vh[st].rearrange("p (h c) -> p h c", c=65)
                nc.vector.tensor_scalar_mul(
                    out=vh3[:, :, 0:64],
                    in0=pss[i][:, :DH].rearrange("p (h c) -> p h c", c=64),
                    scalar1=mv_sb[:, ds(st, 1)],
                )
                nc.vector.tensor_scalar_mul(
                    out=vh3[:, :, 64:65],
                    in0=ones6[:].rearrange("p (h c) -> p h c", c=1),
                    scalar1=mv_sb[:, ds(st, 1)],
                )

        for ci, (off, w) in enumerate(KCH):
            nst = w // P
            for s0 in range(0, nst, 2):
                vproj_sts(vx[ci], off // P, list(range(s0, min(s0 + 2, nst))))

        # q-projection: first q-block now, rest deferred into phase 2
        proj_chunk(q0x, wq_sb, bq_sb, qhT, 0, QCH[0][1])
        pend_qproj = [
            (ch, dt) for ch in range(1, len(QCH)) for dt in range(3)
        ]
        qproj_xt = {}

        def issue_qdef_dma(ch):
            qx = qdef.tile([P, 6 * 512], MMD, name="qx", tag="qx")
            qproj_xt[ch] = qx
            fetch_split(qx, qT, QCH[ch][0], 512)

        # ---- phase 2: attention, head-pair steps ----
        # Each step handles BOTH heads of a pair for one k-chunk: the two
        # scores matmuls live in disjoint PE row groups (base partition 0
        # and 64) so they run concurrently, and share one [128,1024] PSUM
        # tile (head A in cols 0:512, head B in 512:1024) -> one exp per
        # step. Scores run 2 steps ahead of attn@V. Filler work is placed
        # so transient PSUM use never exceeds the free banks: drains right
        # at group start (kc 1,2) free the previous ctx pair early, then
        # q-proj (kc 4,6) and O-proj halves (odd kc>=3) fill PE slack.
        hq = [(pr, qb) for qb in range(NQB) for pr in range(3)]
        steps = [(pr, qb, kc) for (pr, qb) in hq for kc in range(NKT)]
        drain_slots = (1, 2)
        qproj_slots = (4, 6)
        oproj_slots = tuple(k for k in range(3, NKT) if k % 2 == 1)

        ctx_ps = {}
        st_ps = {}
        ot_map = {}

        def scores(pr, qb, kc):
            ps = psA.tile([P, 1024], F32, name="psA", tag="psA")
            for hh in range(2):
                nc.tensor.matmul(
                    ps[:, ts(hh, 512)],
                    lhsT=khT[pr][64 * hh : 64 * hh + 64, ts(kc, P)],
                    rhs=qhT[pr][64 * hh : 64 * hh + 64, ts(qb, QBW)],
                    start=True,
                    stop=True,
                )
            st_ps[(pr, qb, kc)] = ps

        def attnv(pr, qb, kc, pt):
            for hh in range(2):
                h = 2 * pr + hh
                nc.tensor.matmul(
                    ctx_ps[(h, qb)][0:65, :],
                    lhsT=vh[kc][:, ds(65 * h, 65)],
                    rhs=pt[:, ts(hh, 512)],
                    start=(kc == 0),
                    stop=(kc == NKT - 1),
                )

        def drain_many(items):
            """Normalize + store ctx for finished (h, qb) pairs, with the
            per-engine stages interleaved so the DVE/gpsimd chains of the
            different heads overlap instead of serializing."""
            st = []
            for h, qb in items:
                cps = ctx_ps.pop((h, qb))
                # reciprocal_approx_fast can't read partition-offset APs
                # (the custom-DVE encoding drops the partition base), so
                # stage the denominator row at partition 0 first.
                dcp = small.tile([1, QBW], F32, name="dcp", tag="dcp")
                nc.vector.tensor_copy(dcp[:], cps[64:65, :])
                st.append((h, qb, cps, dcp))
            rss = []
            for h, qb, cps, dcp in st:
                rs = small.tile([1, QBW], F32, name="rs", tag="rs")
                nc.vector.reciprocal_approx_fast(rs[:], dcp[:])
                rss.append(rs)
            bcss = []
            for (h, qb, cps, dcp), rs in zip(st, rss):
                bcs = small.tile([64, QBW], F32, name="bcs", tag="bcs")
                nc.gpsimd.partition_broadcast(bcs[:], rs[:])
                bcss.append(bcs)
            for (h, qb, cps, dcp), bcs in zip(st, bcss):
                dt, pb = h // 2, 64 * (h % 2)
                if pb == 0:
                    nc.vector.tensor_tensor(
                        out=cn[dt][0:64, ts(qb, QBW)],
                        in0=cps[0:64, :],
                        in1=bcs[:],
                        op=mybir.AluOpType.mult,
                    )
                else:
                    tmp = small.tile([64, QBW], MMD, name="tmp", tag="tmp")
                    nc.vector.tensor_tensor(
                        out=tmp[:], in0=cps[0:64, :], in1=bcs[:],
                        op=mybir.AluOpType.mult,
                    )
                    nc.sync.dma_start(cn[dt][64:128, ts(qb, QBW)], tmp[:])

        def oproj_half(qc, n0, nw, pool=None):
            if pool is None:
                ps = psB.tile([P, 512], F32, name="psB", tag="psB")
            else:
                ps = pool.tile([P, 1024], F32, name="psA", tag="psA")
            for dt in range(3):
                nc.tensor.matmul(
                    ps[:, :nw],
                    lhsT=cn[dt][:, ts(qc, P)],
                    rhs=wo_sb[:, ds(dt * D + n0, nw)],
                    start=(dt == 0),
                    stop=(dt == 2),
                )
            if n0 == 0:
                ot = outp.tile([P, D], F32, name="ot", tag="ot")
                ot_map[qc] = ot
                nc.vector.tensor_copy(ot[:, 0:512], ps[:, :nw])
            else:
                ot = ot_map.pop(qc)
                nc.vector.tensor_copy(ot[:, 512:768], ps[:, :nw])
                nc.sync.dma_start(out[ts(qc, P), :], ot[:])

        def qproj_sub(ch, dt):
            ps = psB.tile([P, 512], F32, name="psB", tag="psB")
            qx = qproj_xt[ch]
            for c in range(6):
                nc.tensor.matmul(
                    ps[:],
                    lhsT=wq_sb[:, ds(c * DH + dt * P, P)],
                    rhs=qx[:, ds(c * 512, 512)],
                    start=(c == 0),
                    stop=(c == 5),
                )
            nc.vector.tensor_scalar_add(
                out=qhT[dt][:, ds(QCH[ch][0], 512)], in0=ps[:],
                scalar1=bq_sb[:, ds(dt, 1)],
            )
            if dt == 2:
                qproj_xt.pop(ch)

        DEPTH = 2
        pend_drain = []
        pend_oproj = []
        for n, (pr, qb, kc) in enumerate(steps):
            grp = n // NKT
            if kc == 0:
                # lazy bulk DMAs: deferred q chunks at groups 0..; wo at 1
                if grp + 1 < len(QCH):
                    issue_qdef_dma(grp + 1)
                if grp == 1:
                    fetch_cpd(wo_sb, wo, 0, D, nc.sync, csz=D, nch=3)
                for hh in range(2):
                    ctx_ps[(2 * pr + hh, qb)] = psB.tile(
                        [P, QBW], F32, name="psB", tag="psB"
                    )[0:65, :]
            if n < DEPTH:
                scores(*steps[n])
            pt = ppool.tile([P, 1024], MMD, name="pt", tag="pt")
            nc.scalar.activation(
                pt[:], st_ps.pop((pr, qb, kc))[:],
                mybir.ActivationFunctionType.Exp, scale=0.125,
            )
            if n + DEPTH < len(steps):
                scores(*steps[n + DEPTH])
            attnv(pr, qb, kc, pt)
            if kc in drain_slots and pend_drain:
                hd, qd = pend_drain.pop(0)
                drain_many([(hd, qd)])
                if hd == HPC - 1:
                    for qc in range(qd * (QBW // P), (qd + 1) * (QBW // P)):
                        pend_oproj.append((qc, 0, 512))
                        pend_oproj.append((qc, 512, 256))
            elif kc in qproj_slots and pend_qproj and grp >= 1:
                qproj_sub(*pend_qproj.pop(0))
            elif kc in oproj_slots and pend_oproj:
                oproj_half(*pend_oproj.pop(0))
            if kc == NKT - 1:
                pend_drain.extend([(2 * pr, qb), (2 * pr + 1, qb)])
        while pend_drain:
            batch, pend_drain = pend_drain[:2], pend_drain[2:]
            drain_many(batch)
            for hd, qd in batch:
                if hd == HPC - 1:
                    for qc in range(qd * (QBW // P), (qd + 1) * (QBW // P)):
                        pend_oproj.append((qc, 0, 512))
                        pend_oproj.append((qc, 512, 256))
        # tail: alternate psA (idle now) and psB so up to 6 halves in flight
        for i, (qc, n0, nw) in enumerate(pend_oproj):
            oproj_half(qc, n0, nw, pool=psA if i % 2 == 0 else None)

    nc.compile()
    return nc


_NC_CACHE = {}


def _get_nc(S, SK, bf16=True):
    key = (S, SK, bf16)
    if key not in _NC_CACHE:
        _NC_CACHE[key] = build_nc(S, SK, bf16)
    return _NC_CACHE[key]


def _install_ntff_hook():
    try:
        mod = types.ModuleType("antenv.axon_hooks")
        state = {"hook": None}
        mod.set_axon_ntff_profile_hook = lambda h: state.__setitem__("hook", h)
        mod.get_axon_ntff_profile_hook = lambda: state["hook"]
        sys.modules["antenv.axon_hooks"] = mod
        from trn_agent_boot.trn_boot import _ntff_profile_via_ctypes

        mod.set_axon_ntff_profile_hook(
            _ntff_profile_via_ctypes("/opt/axon/libaxon_pjrt.so")
        )
        bass_utils.upload_artifacts = lambda tmpdir: "local://" + tmpdir
        return state["hook"] is not None
    except Exception:
        return False


def run_cores(in_maps, S=2048, SK=1152, bf16=True, profile=False):
    nc = _get_nc(S, SK, bf16)
    trace = bool(profile) and _install_ntff_hook()
    res = bass_utils.run_bass_kernel_spmd(
        nc, in_maps, core_ids=list(range(len(in_maps))), trace=trace
    )
    return res


def make_in_maps(q, k, v, mask, Wq, bq, Wk, bk, Wv, Wo, bf16=True):
    B, S, _ = q.shape
    mmd = ml_dtypes.bfloat16 if bf16 else np.float32
    mbool = np.asarray(mask).reshape(B, S)
    keep = [np.nonzero(~mbool[b])[0] for b in range(B)]
    nmax = max((len(ix) for ix in keep), default=1)
    SK = max(128, ((int(nmax) + 127) // 128) * 128)

    qT = np.ascontiguousarray(
        np.asarray(q, np.float32).transpose(0, 2, 1)).astype(mmd)
    kT33 = np.asarray(k, np.float32).transpose(0, 2, 1)
    vT33 = np.asarray(v, np.float32).transpose(0, 2, 1)
    kTc = np.zeros((B, D, SK), np.float32)
    vTc = np.zeros((B, D, SK), np.float32)
    mvec = np.zeros((B, SK), np.float32)
    for b in range(B):
        nb = len(keep[b])
        kTc[b, :, :nb] = kT33[b][:, keep[b]]
        vTc[b, :, :nb] = vT33[b][:, keep[b]]
        mvec[b, :nb] = 1.0
    kTc = kTc.astype(mmd)
    vTc = vTc.astype(mmd)
    Wq, Wk, Wv, Wo = (np.asarray(a, np.float32) for a in (Wq, Wk, Wv, Wo))
    bq, bk = np.asarray(bq, np.float32), np.asarray(bk, np.float32)
    in_maps = []
    for b in range(B):
        for half in range(2):
            hs = slice(DH * half, DH * (half + 1))
            in_maps.append(
                {
                    "qT": qT[b],
                    "kT": kTc[b],
                    "vT": vTc[b],
                    "wq": np.ascontiguousarray(Wq[:, hs]).astype(mmd),
                    "wk": np.ascontiguousarray(Wk[:, hs]).astype(mmd),
                    "wv": np.ascontiguousarray(Wv[:, hs]).astype(mmd),
                    "wo": np.ascontiguousarray(Wo[hs, :]).astype(mmd),
                    "bq": np.ascontiguousarray(bq[hs]).reshape(DH, 1),
                    "bk": np.ascontiguousarray(bk[hs]).reshape(DH, 1),
                    "mv": np.ascontiguousarray(mvec[b]).reshape(-1, 1),
                }
            )
    return in_maps, SK


def kernel(q, k, v, mask, Wq, bq, Wk, bk, Wv, bv, Wo, bo):
    q = np.asarray(q, np.float32)
    B, S, _ = q.shape
    bf16 = os.environ.get("BASS_PRECISE") != "1"
    in_maps, SK = make_in_maps(
        q, k, v, mask, Wq, bq, Wk, bk, Wv, Wo, bf16=bf16
    )
    res = run_cores(
        in_maps, S=S, SK=SK, bf16=bf16,
        profile=os.environ.get("BASS_PROFILE") == "1",
    )
    if os.environ.get("BASS_PROFILE") == "1" and res.exec_time_ns is not None:
        print(f"HW exec time: {res.exec_time_ns} ns")
    cvec = (
        np.asarray(bv, np.float32) @ np.asarray(Wo, np.float32)
        + np.asarray(bo, np.float32)
    )
    out = np.empty((B, S, D), np.float32)
    for b in range(B):
        out[b] = res.results[2 * b]["out"] + res.results[2 * b + 1]["out"] + cvec
    return out


# revision 19
# speedup vs baseline: 1.2541x; 1.0005x over previous
"""MultiHeadAttention TRN2 Bass kernel.

Problem: B=4, S=2048, D=768, H=12 heads (DK=64).
Sharding: 8 cores = (batch b in 0..3) x (head-half in 0..1); each core
computes 6 heads of one batch element end-to-end (tensor-parallel over
heads within a batch). Host pre-transposes activations to [D, S] (and
casts to bf16 in the default fast path), slices projection weights per
head-half, and sums the two partial outputs per batch (+ bv@Wo + bo
correction, exact because softmax rows sum to 1).

Key compaction: the boolean mask drops ~half the key positions, so the
host gathers the unmasked keys to the front of kT/vT (padded with zeros
to a multiple of 128, SK columns total) and sets mv=1 for real keys /
0 for padding. The padded columns flow through the same mask-fold math
(vh_aug *= mv) the full kernel used, so the result is exact while the
score/exp/attn@V/k-proj/v-proj work drops to SK/S of the dense cost.

On-core math:
  qh^T[384, S]: lhsT=Wq tile [Din,dout], rhs=q^T tile [Din,s] (+bq in drain)
  kh^T likewise over SK; vh natural [SK, 390] via lhsT=v^T tile, rhs=Wv:
    vh_aug[s, 65j..65j+64] = [m(s)*vh_head_j(s, :), m(s)]  (mask fold)
  S^T[k, q] = kh_head^T.T @ qh_head^T  (contraction d=64)
  P^T = exp(S^T * 0.125)               (ACT, fused scale, no max-sub)
  ctx_aug^T[0:65, q] += vh_aug_j[kc].T @ P^T[kc]  over k-chunks
    rows 0..63 = unnormalized ctx^T, row 64 = softmax denominator
  rs = recip-approx(denom); bcast via gpsimd partition_broadcast;
  cn = ctx^T * rs   (drains deferred one (h,qb) iteration so the PE
                     pipeline never waits on the reciprocal chain)
  out[q, 768] = sum_dt cn[dt].T @ Wo tiles  (per 128-q chunk, split in
  two PSUM-bank-sized halves so transient PSUM stays at 1 bank)

Bandwidth notes: every input stream is fetched with ONE 3D-strided DMA
per chunk ([768, w] DRAM block -> [128, 6*w] SBUF tile, p-c-s order) so
phase 1 issues ~13 DMA instructions instead of ~45 (each costs ~600ns
of queue issue time). Projection dt-chains are interleaved across three
PSUM banks to keep the PE pipeline full.
"""

import os
import sys
import types
from contextlib import ExitStack

import ml_dtypes
import numpy as np

import concourse.bacc as bacc
import concourse.bass as bass
import concourse.mybir as mybir
import concourse.tile as tile
from concourse import bass_utils
from concourse.bass import ts, ds

F32 = mybir.dt.float32
F32R = mybir.dt.float32r
BF16 = mybir.dt.bfloat16

D = 768        # model dim
DH = 384       # per-core head dim (6 heads x 64)
HPC = 6        # heads per core
VW = HPC * 65  # vh_aug free width (390)


def _chunks(total, w=512):
    out = []
    off = 0
    while off < total:
        cw = min(w, total - off)
        out.append((off, cw))
        off += cw
    return out


def build_nc(S=2048, SK=1152, bf16=True):
    nc = bacc.Bacc("TRN2", target_bir_lowering=False, debug=False)

    MMD = BF16 if bf16 else F32R    # matmul operand dtype
    QBW = min(512, S)               # attention q-block width
    NKT = SK // 128                 # 128-wide k tiles
    NQB = S // QBW                  # q blocks
    KCH = _chunks(SK)               # k/v projection chunks (<=512 wide)
    QCH = _chunks(S)                # q projection chunks (512 wide)

    qT = nc.dram_tensor("qT", [D, S], MMD, kind="ExternalInput").ap()
    kT = nc.dram_tensor("kT", [D, SK], MMD, kind="ExternalInput").ap()
    vT = nc.dram_tensor("vT", [D, SK], MMD, kind="ExternalInput").ap()
    wq = nc.dram_tensor("wq", [D, DH], MMD, kind="ExternalInput").ap()
    wk = nc.dram_tensor("wk", [D, DH], MMD, kind="ExternalInput").ap()
    wv = nc.dram_tensor("wv", [D, DH], MMD, kind="ExternalInput").ap()
    wo = nc.dram_tensor("wo", [DH, D], MMD, kind="ExternalInput").ap()
    bq = nc.dram_tensor("bq", [DH, 1], F32, kind="ExternalInput").ap()
    bk = nc.dram_tensor("bk", [DH, 1], F32, kind="ExternalInput").ap()
    mv = nc.dram_tensor("mv", [SK, 1], F32, kind="ExternalInput").ap()
    out = nc.dram_tensor("out", [S, D], F32, kind="ExternalOutput").ap()

    with tile.TileContext(nc) as tc, ExitStack() as ctx:
        P = 128
        wpool = ctx.enter_context(tc.tile_pool(name="w", bufs=1))
        xin = ctx.enter_context(
            tc.tile_pool(name="xin", bufs=2 * len(KCH) + 1)
        )
        qdef = ctx.enter_context(tc.tile_pool(name="qdef", bufs=3))
        persist = ctx.enter_context(tc.tile_pool(name="persist", bufs=1))
        ppool = ctx.enter_context(tc.tile_pool(name="p", bufs=3))
        small = ctx.enter_context(tc.tile_pool(name="small", bufs=2))
        outp = ctx.enter_context(tc.tile_pool(name="outp", bufs=3))
        psA = ctx.enter_context(tc.tile_pool(name="psA", bufs=2, space="PSUM"))
        psB = ctx.enter_context(tc.tile_pool(name="psB", bufs=4, space="PSUM"))

        def fetch_cpd(dst, src, coff, w, eng, csz=512, nch=6):
            """One 3D DMA: DRAM [nch*P, w] block -> SBUF [P, nch blocks of csz]."""
            d3 = dst[:].rearrange("p (c s) -> p c s", s=csz)[:, :, :w]
            s3 = src[ds(0, nch * P), ds(coff, w)].rearrange(
                "(c p) s -> p c s", p=P
            )
            eng.dma_start(d3, s3)

        # ---- weights + all phase-1 input streams, one DMA per chunk,
        # issued upfront in consumption order across both bulk queues ----
        wk_sb = wpool.tile([P, 6 * DH], MMD, tag="wk")
        wq_sb = wpool.tile([P, 6 * DH], MMD, tag="wq")
        wv_sb = wpool.tile([P, 6 * DH], MMD, tag="wv")
        wo_sb = wpool.tile([P, 3 * D], MMD, tag="wo")
        kx = [xin.tile([P, 6 * 512], MMD, name="xin", tag="xin") for _ in KCH]
        vx = [xin.tile([P, 6 * 512], MMD, name="xin", tag="xin") for _ in KCH]
        q0x = xin.tile([P, 6 * 512], MMD, name="xin", tag="xin")

        def fetch_split(dst, src, coff, w):
            """Bulk chunk fetch: one 2D DMA per 128-row block, alternating
            queues, so the transfer spreads across DMA engines."""
            for c in range(6):
                (nc.sync if c % 2 == 0 else nc.gpsimd).dma_start(
                    dst[:, ds(c * 512, w)], src[ts(c, P), ds(coff, w)]
                )

        fetch_cpd(wk_sb, wk, 0, DH, nc.sync, csz=DH)
        fetch_cpd(wv_sb, wv, 0, DH, nc.gpsimd, csz=DH)
        for ci in range(len(KCH)):
            fetch_split(kx[ci], kT, KCH[ci][0], KCH[ci][1])
        for ci in range(len(KCH)):
            fetch_split(vx[ci], vT, KCH[ci][0], KCH[ci][1])
        fetch_cpd(wq_sb, wq, 0, DH, nc.gpsimd, csz=DH)
        fetch_split(q0x, qT, 0, 512)

        # small tensors on the scalar queue (idle during phase 1)
        bq_sb = wpool.tile([P, 3], F32, tag="bq")
        bk_sb = wpool.tile([P, 3], F32, tag="bk")
        mv_sb = wpool.tile([P, NKT], F32, tag="mv")
        nc.scalar.dma_start(
            bq_sb[:].rearrange("p (t u) -> p t u", u=1),
            bq[ds(0, DH), :].rearrange("(t p) u -> p t u", p=P),
        )
        nc.scalar.dma_start(
            bk_sb[:].rearrange("p (t u) -> p t u", u=1),
            bk[ds(0, DH), :].rearrange("(t p) u -> p t u", p=P),
        )
        nc.scalar.dma_start(
            mv_sb[:].rearrange("p (t u) -> p t u", u=1),
            mv[ds(0, SK), :].rearrange("(t p) u -> p t u", p=P),
        )
        ones6 = wpool.tile([P, HPC], F32, tag="ones6")
        nc.vector.memset(ones6[:], 1.0)

        # ---- persistent activations ----
        khT = [persist.tile([P, SK], MMD, name=f"khT{t}", tag=f"khT{t}") for t in range(3)]
        qhT = [persist.tile([P, S], MMD, name=f"qhT{t}", tag=f"qhT{t}") for t in range(3)]
        vh = [persist.tile([P, VW], MMD, name=f"vh{st}", tag=f"vh{st}") for st in range(NKT)]
        cn = [persist.tile([P, S], MMD, name=f"cn{t}", tag=f"cn{t}") for t in range(3)]

        # ---- phase 1: projections. dt-chains interleave across three PSUM
        # banks; q chunks 1.. are deferred into the attention stream ----
        def proj_chunk(xt, wsb, bsb, dst, off, w):
            ps3 = [psB.tile([P, 512], F32, name="psB", tag="psB") for _ in range(3)]
            for c in range(6):
                for dt in range(3):
                    nc.tensor.matmul(
                        ps3[dt][:, :w],
                        lhsT=wsb[:, ds(c * DH + dt * P, P)],
                        rhs=xt[:, ds(c * 512, w)],
                        start=(c == 0),
                        stop=(c == 5),
                    )
            for dt in range(3):
                nc.vector.tensor_scalar_add(
                    out=dst[dt][:, ds(off, w)], in0=ps3[dt][:, :w],
                    scalar1=bsb[:, ds(dt, 1)],
                )

        for ci, (off, w) in enumerate(KCH):
            proj_chunk(kx[ci], wk_sb, bk_sb, khT, off, w)

        # v-projection: st chains pair-interleaved across two PSUM banks
        def vproj_sts(vt, base_st, sjs):
            pss = [psB.tile([P, 512], F32, name="psB", tag="psB") for _ in sjs]
            for c in range(6):
                for i, sj in enumerate(sjs):
                    nc.tensor.matmul(
                        pss[i][:, :DH],
                        lhsT=vt[:, ds(c * 512 + sj * P, P)],
                        rhs=wv_sb[:, ds(c * DH, DH)],
                        start=(c == 0),
                        stop=(c == 5),
                    )
            for i, sj in enumerate(sjs):
                st = base_st + sj
                vh3 = vh[st].rearrange("p (h c) -> p h c", c=65)
                nc.vector.tensor_scalar_mul(
                    out=vh3[:, :, 0:64],
                    in0=pss[i][:, :DH].rearrange("p (h c) -> p h c", c=64),
                    scalar1=mv_sb[:, ds(st, 1)],
                )
                nc.vector.tensor_scalar_mul(
                    out=vh3[:, :, 64:65],
                    in0=ones6[:].rearrange("p (h c) -> p h c", c=1),
                    scalar1=mv_sb[:, ds(st, 1)],
                )

        for ci, (off, w) in enumerate(KCH):
            nst = w // P
            for s0 in range(0, nst, 2):
                vproj_sts(vx[ci], off // P, list(range(s0, min(s0 + 2, nst))))

        # q-projection: first q-block now, rest deferred into phase 2
        proj_chunk(q0x, wq_sb, bq_sb, qhT, 0, QCH[0][1])
        pend_qproj = [
            (ch, dt, half)
            for ch in range(1, len(QCH))
            for dt in range(3)
            for half in range(2)
        ]
        qproj_xt = {}

        def issue_qdef_dma(ch):
            qx = qdef.tile([P, 6 * 512], MMD, name="qx", tag="qx")
            qproj_xt[ch] = qx
            fetch_split(qx, qT, QCH[ch][0], 512)

        # ---- phase 2: attention, head-pair steps ----
        # Each step handles BOTH heads of a pair for one k-chunk: the two
        # scores matmuls live in disjoint PE row groups (base partition 0
        # and 64) so they run concurrently, and share one [128,1024] PSUM
        # tile (head A in cols 0:512, head B in 512:1024) -> one exp per
        # step. Scores run 2 steps ahead of attn@V. Filler work is placed
        # so transient PSUM use never exceeds the free banks: drains right
        # at group start (kc 1,2) free the previous ctx pair early, then
        # q-proj (kc 4,6) and O-proj halves (odd kc>=3) fill PE slack.
        hq = [(pr, qb) for qb in range(NQB) for pr in range(3)]
        steps = [(pr, qb, kc) for (pr, qb) in hq for kc in range(NKT)]
        drain_slots = (1, 2)
        qproj_slots = (4, 6)
        oproj_slots = tuple(k for k in range(3, NKT) if k % 2 == 1)

        ctx_ps = {}
        st_ps = {}
        ot_map = {}

        def scores(pr, qb, kc):
            ps = psA.tile([P, 1024], F32, name="psA", tag="psA")
            for hh in range(2):
                nc.tensor.matmul(
                    ps[:, ts(hh, 512)],
                    lhsT=khT[pr][64 * hh : 64 * hh + 64, ts(kc, P)],
                    rhs=qhT[pr][64 * hh : 64 * hh + 64, ts(qb, QBW)],
                    start=True,
                    stop=True,
                )
            st_ps[(pr, qb, kc)] = ps

        def attnv(pr, qb, kc, pt):
            for hh in range(2):
                h = 2 * pr + hh
                nc.tensor.matmul(
                    ctx_ps[(h, qb)][0:65, :],
                    lhsT=vh[kc][:, ds(65 * h, 65)],
                    rhs=pt[:, ts(hh, 512)],
                    start=(kc == 0),
                    stop=(kc == NKT - 1),
                )

        def drain_many(items):
            """Normalize + store ctx for finished (h, qb) pairs, with the
            per-engine stages interleaved so the DVE/gpsimd chains of the
            different heads overlap instead of serializing."""
            st = []
            for h, qb in items:
                cps = ctx_ps.pop((h, qb))
                # reciprocal_approx_fast can't read partition-offset APs
                # (the custom-DVE encoding drops the partition base), so
                # stage the denominator row at partition 0 first.
                dcp = small.tile([1, QBW], F32, name="dcp", tag="dcp")
                nc.vector.tensor_copy(dcp[:], cps[64:65, :])
                st.append((h, qb, cps, dcp))
            rss = []
            for h, qb, cps, dcp in st:
                rs = small.tile([1, QBW], F32, name="rs", tag="rs")
                nc.vector.reciprocal_approx_fast(rs[:], dcp[:])
                rss.append(rs)
            bcss = []
            for (h, qb, cps, dcp), rs in zip(st, rss):
                bcs = small.tile([64, QBW], F32, name="bcs", tag="bcs")
                nc.gpsimd.partition_broadcast(bcs[:], rs[:])
                bcss.append(bcs)
            for (h, qb, cps, dcp), bcs in zip(st, bcss):
                dt, pb = h // 2, 64 * (h % 2)
                if pb == 0:
                    nc.vector.tensor_tensor(
                        out=cn[dt][0:64, ts(qb, QBW)],
                        in0=cps[0:64, :],
                        in1=bcs[:],
                        op=mybir.AluOpType.mult,
                    )
                else:
                    tmp = small.tile([64, QBW], MMD, name="tmp", tag="tmp")
                    nc.vector.tensor_tensor(
                        out=tmp[:], in0=cps[0:64, :], in1=bcs[:],
                        op=mybir.AluOpType.mult,
                    )
                    nc.sync.dma_start(cn[dt][64:128, ts(qb, QBW)], tmp[:])

        def oproj_half(qc, n0, nw, pool=None):
            if pool is None:
                ps = psB.tile([P, 512], F32, name="psB", tag="psB")
            else:
                ps = pool.tile([P, 1024], F32, name="psA", tag="psA")
            for dt in range(3):
                nc.tensor.matmul(
                    ps[:, :nw],
                    lhsT=cn[dt][:, ts(qc, P)],
                    rhs=wo_sb[:, ds(dt * D + n0, nw)],
                    start=(dt == 0),
                    stop=(dt == 2),
                )
            if n0 == 0:
                ot = outp.tile([P, D], F32, name="ot", tag="ot")
                ot_map[qc] = ot
                nc.vector.tensor_copy(ot[:, 0:512], ps[:, :nw])
            else:
                ot = ot_map.pop(qc)
                nc.vector.tensor_copy(ot[:, 512:768], ps[:, :nw])
                (nc.sync if qc % 2 == 0 else nc.gpsimd).dma_start(
                    out[ts(qc, P), :], ot[:]
                )

        qp_hold = {}

        def qproj_sub(ch, dt, half):
            # half-chains (3 matmuls) keep PE filler bursts under ~1us so
            # the scores->exp pipeline never starves the ACT engine
            if half == 0:
                ps = psB.tile([P, 512], F32, name="psB", tag="psB")
                qp_hold[(ch, dt)] = ps
            else:
                ps = qp_hold.pop((ch, dt))
            qx = qproj_xt[ch]
            for c in range(3 * half, 3 * half + 3):
                nc.tensor.matmul(
                    ps[:],
                    lhsT=wq_sb[:, ds(c * DH + dt * P, P)],
                    rhs=qx[:, ds(c * 512, 512)],
                    start=(c == 0),
                    stop=(c == 5),
                )
            if half == 1:
                nc.vector.tensor_scalar_add(
                    out=qhT[dt][:, ds(QCH[ch][0], 512)], in0=ps[:],
                    scalar1=bq_sb[:, ds(dt, 1)],
                )
                if dt == 2:
                    qproj_xt.pop(ch)

        DEPTH = 2
        pend_drain = []
        pend_oproj = []
        for n, (pr, qb, kc) in enumerate(steps):
            grp = n // NKT
            if kc == 0:
                # lazy bulk DMAs: deferred q chunks at groups 0..; wo at 1
                if grp + 1 < len(QCH):
                    issue_qdef_dma(grp + 1)
                if grp == 1:
                    fetch_cpd(wo_sb, wo, 0, D, nc.sync, csz=D, nch=3)
                for hh in range(2):
                    ctx_ps[(2 * pr + hh, qb)] = psB.tile(
                        [P, QBW], F32, name="psB", tag="psB"
                    )[0:65, :]
            if n < DEPTH:
                scores(*steps[n])
            pt = ppool.tile([P, 1024], MMD, name="pt", tag="pt")
            nc.scalar.activation(
                pt[:], st_ps.pop((pr, qb, kc))[:],
                mybir.ActivationFunctionType.Exp, scale=0.125,
            )
            if n + DEPTH < len(steps):
                scores(*steps[n + DEPTH])
            attnv(pr, qb, kc, pt)
            if kc in drain_slots and pend_drain:
                hd, qd = pend_drain.pop(0)
                drain_many([(hd, qd)])
                if hd == HPC - 1:
                    for qc in range(qd * (QBW // P), (qd + 1) * (QBW // P)):
                        pend_oproj.append((qc, 0, 512))
                        pend_oproj.append((qc, 512, 256))
            elif kc in qproj_slots and pend_qproj and grp >= 1:
                qproj_sub(*pend_qproj.pop(0))
            elif kc in oproj_slots and pend_oproj:
                oproj_half(*pend_oproj.pop(0))
            if kc == NKT - 1:
                pend_drain.extend([(2 * pr, qb), (2 * pr + 1, qb)])
        while pend_drain:
            batch, pend_drain = pend_drain[:2], pend_drain[2:]
            drain_many(batch)
            for hd, qd in batch:
                if hd == HPC - 1:
                    for qc in range(qd * (QBW // P), (qd + 1) * (QBW // P)):
                        pend_oproj.append((qc, 0, 512))
                        pend_oproj.append((qc, 512, 256))
        # tail: alternate psA (idle now) and psB so up to 6 halves in flight
        for i, (qc, n0, nw) in enumerate(pend_oproj):
            oproj_half(qc, n0, nw, pool=psA if i % 2 == 0 else None)

    nc.compile()
    return nc


_NC_CACHE = {}


def _get_nc(S, SK, bf16=True):
    key = (S, SK, bf16)
    if key not in _NC_CACHE:
        _NC_CACHE[key] = build_nc(S, SK, bf16)
    return _NC_CACHE[key]


def _install_ntff_hook():
    try:
        mod = types.ModuleType("antenv.axon_hooks")
        state = {"hook": None}
        mod.set_axon_ntff_profile_hook = lambda h: state.__setitem__("hook", h)
        mod.get_axon_ntff_profile_hook = lambda: state["hook"]
        sys.modules["antenv.axon_hooks"] = mod
        from trn_agent_boot.trn_boot import _ntff_profile_via_ctypes

        mod.set_axon_ntff_profile_hook(
            _ntff_profile_via_ctypes("/opt/axon/libaxon_pjrt.so")
        )
        bass_utils.upload_artifacts = lambda tmpdir: "local://" + tmpdir
        return state["hook"] is not None
    except Exception:
        return False


def run_cores(in_maps, S=2048, SK=1152, bf16=True, profile=False):
    nc = _get_nc(S, SK, bf16)
    trace = bool(profile) and _install_ntff_hook()
    res = bass_utils.run_bass_kernel_spmd(
        nc, in_maps, core_ids=list(range(len(in_maps))), trace=trace
    )
    return res


def make_in_maps(q, k, v, mask, Wq, bq, Wk, bk, Wv, Wo, bf16=True):
    B, S, _ = q.shape
    mmd = ml_dtypes.bfloat16 if bf16 else np.float32
    mbool = np.asarray(mask).reshape(B, S)
    keep = [np.nonzero(~mbool[b])[0] for b in range(B)]
    nmax = max((len(ix) for ix in keep), default=1)
    SK = max(128, ((int(nmax) + 127) // 128) * 128)

    qT = np.ascontiguousarray(
        np.asarray(q, np.float32).transpose(0, 2, 1)).astype(mmd)
    kT33 = np.asarray(k, np.float32).transpose(0, 2, 1)
    vT33 = np.asarray(v, np.float32).transpose(0, 2, 1)
    kTc = np.zeros((B, D, SK), np.float32)
    vTc = np.zeros((B, D, SK), np.float32)
    mvec = np.zeros((B, SK), np.float32)
    for b in range(B):
        nb = len(keep[b])
        kTc[b, :, :nb] = kT33[b][:, keep[b]]
        vTc[b, :, :nb] = vT33[b][:, keep[b]]
        mvec[b, :nb] = 1.0
    kTc = kTc.astype(mmd)
    vTc = vTc.astype(mmd)
    Wq, Wk, Wv, Wo = (np.asarray(a, np.float32) for a in (Wq, Wk, Wv, Wo))
    bq, bk = np.asarray(bq, np.float32), np.asarray(bk, np.float32)
    in_maps = []
    for b in range(B):
        for half in range(2):
            hs = slice(DH * half, DH * (half + 1))
            in_maps.append(
                {
                    "qT": qT[b],
                    "kT": kTc[b],
                    "vT": vTc[b],
                    "wq": np.ascontiguousarray(Wq[:, hs]).astype(mmd),
                    "wk": np.ascontiguousarray(Wk[:, hs]).astype(mmd),
                    "wv": np.ascontiguousarray(Wv[:, hs]).astype(mmd),
                    "wo": np.ascontiguousarray(Wo[hs, :]).astype(mmd),
                    "bq": np.ascontiguousarray(bq[hs]).reshape(DH, 1),
                    "bk": np.ascontiguousarray(bk[hs]).reshape(DH, 1),
                    "mv": np.ascontiguousarray(mvec[b]).reshape(-1, 1),
                }
            )
    return in_maps, SK


def kernel(q, k, v, mask, Wq, bq, Wk, bk, Wv, bv, Wo, bo):
    q = np.asarray(q, np.float32)
    B, S, _ = q.shape
    bf16 = os.environ.get("BASS_PRECISE") != "1"
    in_maps, SK = make_in_maps(
        q, k, v, mask, Wq, bq, Wk, bk, Wv, Wo, bf16=bf16
    )
    res = run_cores(
        in_maps, S=S, SK=SK, bf16=bf16,
        profile=os.environ.get("BASS_PROFILE") == "1",
    )
    if os.environ.get("BASS_PROFILE") == "1" and res.exec_time_ns is not None:
        print(f"HW exec time: {res.exec_time_ns} ns")
    cvec = (
        np.asarray(bv, np.float32) @ np.asarray(Wo, np.float32)
        + np.asarray(bo, np.float32)
    )
    out = np.empty((B, S, D), np.float32)
    for b in range(B):
        out[b] = res.results[2 * b]["out"] + res.results[2 * b + 1]["out"] + cvec
    return out
